# revision 7
# baseline (speedup 1.0000x reference)
"""SSIM-based loss kernel for Trainium2 (8 NeuronCores, data-parallel over batch).

Computes: loss = 1 - (1 + mean(SSIM(sigmoid(seg), sigmoid(edge)))) / 2
for seg, edge of shape [32, 1, 512, 512] fp32, SSIM with a 7x7 gaussian
window (sigma=1.5), SAME zero-padding, C1=0.01^2, C2=0.03^2.

Sharding: batch dim across 8 cores (4 images each). Each core returns
per-partition partial sums of the ssim map; the host reduces and forms the
scalar loss.

v2: all matmuls in bf16 (fp32 matmul runs as 2 half-speed passes on trn2's
PE; bf16 is ~4x faster and also halves LDWEIGHTS cost via FWL). All
pointwise intermediates bf16 (DVE 2x mode for 16-bit SBUF operands). PSUM
readout (the 1x-rate bottleneck) is split across ACT and DVE. The final
multiply+reduce is fused into one scalar_tensor_tensor with accum_out.

Per-core algorithm (P/M basis, separable blur on the tensor engine):
  s = sigmoid(seg), e = sigmoid(edge)         [ACT, f32->bf16]
  P = s+e, M = s-e, P2 = P^2, M2 = M^2        [DVE, bf16]
  step-1 (row blur, transposing): z* = rowblur(*) via image-chunk-stationary
    matmuls; z PSUM f32 -> SBUF bf16 copies split ACT/DVE.
  step-2 (col blur): band-stationary matmuls:
    pa = colblur(zP)/sqrt2, pb = colblur(zM)/sqrt2
    pu = (colblur(zP2)+colblur(zM2))/2, pv = (colblur(zP2)-colblur(zM2))/2
  x = pa^2, y = pb^2                          [ACT Square from PSUM]
  tu = pu + (C1+C2), tv = pv + (C1+C2)        [ACT Identity+bias from PSUM]
  alpha = x - y + C1        (= 2 mu1 mu2 + C1)
  beta  = x + y + C1        (= mu1^2 + mu2^2 + C1)
  gamma = tv - alpha        (= 2 sigma12 + C2)
  delta = tu - beta         (= sigma1^2 + sigma2^2 + C2)
  ssim  = (alpha*gamma) / (beta*delta); sum via fused STT accum.
"""

import numpy as np
import ml_dtypes

import concourse.bass as bass
import concourse.bacc as bacc
import concourse.tile as tile
import concourse.mybir as mybir
from concourse.bass_utils import run_bass_kernel_spmd

WS = 7
HW = WS // 2
SIGMA = 1.5
C1 = 0.01 ** 2
C2 = 0.03 ** 2
K12 = float(C1 + C2)

N_CORES = 8
IMG = 512
P = 128
PER_CORE = 4

# halo chunking: out regions [O[c], O[c+1]), input rows [R[c], R[c]+128)
O = [0, 122, 244, 366, 488, 512]
R = [0, 119, 241, 363, 384]
NC5 = 5
FD5 = NC5 * IMG  # 2560

F32 = mybir.dt.float32
BF16 = mybir.dt.bfloat16
AF = mybir.ActivationFunctionType
OP = mybir.AluOpType
BF = ml_dtypes.bfloat16


def _gauss():
    x = np.arange(WS, dtype=np.float64)
    g = np.exp(-((x - HW) ** 2) / (2.0 * SIGMA ** 2))
    return g / g.sum()


def _band_tiles(scale):
    """B_c[r, j] = g[(O[c]+j) - (R[c]+r)] for tap offsets in [-3,3], zero
    otherwise. Serves as step-1 moving operand and step-2 stationary."""
    g = _gauss() * scale
    tiles = []
    for c in range(NC5):
        w = O[c + 1] - O[c]
        t = np.zeros((P, w), dtype=np.float64)
        for r in range(P):
            i = R[c] + r
            for j in range(w):
                d = (O[c] + j) - i
                if -HW <= d <= HW:
                    t[r, j] = g[d + HW]
        tiles.append(t.astype(np.float32))
    return tiles


_CACHE = {}


def _build():
    if "nc" in _CACHE:
        return _CACHE["nc"]

    nc = bacc.Bacc(None)

    seg_d = nc.dram_tensor("seg", [PER_CORE, IMG, IMG], F32, kind="ExternalInput")
    edge_d = nc.dram_tensor("edge", [PER_CORE, IMG, IMG], F32, kind="ExternalInput")
    out_d = nc.dram_tensor("out", [P, 1], F32, kind="ExternalOutput")

    # Band variants: 0: step1 (scale 1); 1: mu pipes (1/sqrt2); 2: +1/2; 3: -1/2
    variants = [1.0, 1.0 / np.sqrt(2.0), 0.5, -0.5]
    packed, offsets = [], []
    col = 0
    for v in variants:
        offs = []
        for t in _band_tiles(v):
            offs.append((col, t.shape[1]))
            packed.append(t)
            col += t.shape[1]
        offsets.append(offs)
    band_np = np.concatenate(packed, axis=1).astype(BF)  # [128, 2048] bf16
    band_d = nc.inline_tensor(band_np, name="band")

    with tile.TileContext(nc) as tc:
        with (
            tc.tile_pool(name="const", bufs=1) as constp,
            tc.tile_pool(name="io", bufs=2) as iop,
            tc.tile_pool(name="sig", bufs=2) as sigp,
            tc.tile_pool(name="maps", bufs=2) as mapp,
            tc.tile_pool(name="zmaps", bufs=2) as zp,
            tc.tile_pool(name="post", bufs=1) as postp,
            tc.tile_pool(name="acc", bufs=1) as accp,
            tc.tile_pool(name="psz", bufs=1, space="PSUM") as psz,
            tc.tile_pool(name="ps2", bufs=1, space="PSUM") as ps2,
        ):
            band = constp.tile([P, band_np.shape[1]], BF16)
            nc.sync.dma_start(band[:], band_d[:])

            def band_ap(v, c):
                c0, w = offsets[v][c]
                return band[:, c0:c0 + w], w

            partials = accp.tile([P, PER_CORE * NC5], F32)
            nc.vector.memset(partials[:], 0.0)
            k12c = constp.tile([P, 1], F32)
            nc.vector.memset(k12c[:], K12)

            for b in range(PER_CORE):
                sg = iop.tile([P, NC5, IMG], F32, tag="sg")
                ed = iop.tile([P, NC5, IMG], F32, tag="ed")
                for c in range(NC5):
                    nc.sync.dma_start(sg[:, c, :], seg_d[b, R[c]:R[c] + P, :])
                    nc.sync.dma_start(ed[:, c, :], edge_d[b, R[c]:R[c] + P, :])

                st = sigp.tile([P, NC5, IMG], BF16, tag="st")
                et = sigp.tile([P, NC5, IMG], BF16, tag="et")
                nc.scalar.activation(st[:], sg[:], AF.Sigmoid)
                nc.scalar.activation(et[:], ed[:], AF.Sigmoid)

                sf = st[:].rearrange("p c w -> p (c w)")
                ef = et[:].rearrange("p c w -> p (c w)")
                Pt = mapp.tile([P, NC5, IMG], BF16, tag="P")
                Mt = mapp.tile([P, NC5, IMG], BF16, tag="M")
                nc.vector.tensor_tensor(Pt[:].rearrange("p c w -> p (c w)"), sf, ef, OP.add)
                nc.vector.tensor_tensor(Mt[:].rearrange("p c w -> p (c w)"), sf, ef, OP.subtract)
                P2t = mapp.tile([P, NC5, IMG], BF16, tag="P2")
                M2t = mapp.tile([P, NC5, IMG], BF16, tag="M2")
                pf = Pt[:].rearrange("p c w -> p (c w)")
                mf = Mt[:].rearrange("p c w -> p (c w)")
                nc.vector.tensor_tensor(P2t[:].rearrange("p c w -> p (c w)"), pf, pf, OP.mult)
                nc.vector.tensor_tensor(M2t[:].rearrange("p c w -> p (c w)"), mf, mf, OP.mult)

                # ---- blur step 1: z[w, j] (transposed, halo layout along w)
                # image-chunk stationary, band moving; PSUM f32 -> SBUF bf16.
                # Copies split: zP,zM on ACT; zP2,zM2 on DVE.
                srcs = [("zP", Pt, "act"), ("zM", Mt, "act"),
                        ("zP2", P2t, "dve"), ("zM2", M2t, "dve")]
                zt = {}
                for name, src, eng in srcs:
                    z = zp.tile([P, NC5, IMG], BF16, tag=name)
                    zt[name] = z
                    for k in range(NC5):
                        pz = psz.tile([P, IMG], F32, tag="pz" + name)
                        for c in range(NC5):
                            rhs, w = band_ap(0, c)
                            nc.tensor.matmul(
                                pz[:, O[c]:O[c + 1]],
                                src[:, c, R[k]:R[k] + P],
                                rhs,
                                start=(c == 0),
                                stop=(c == NC5 - 1),
                            )
                        if eng == "act":
                            nc.scalar.copy(z[:, k, :], pz[:])
                        else:
                            nc.vector.tensor_copy(z[:, k, :], pz[:])

                # ---- blur step 2 + PSUM readout. Outputs transposed
                # ([wo, j]) - irrelevant for the mean reduction.
                xt = postp.tile([P, NC5, IMG], BF16, tag="xt")
                yt = postp.tile([P, NC5, IMG], BF16, tag="yt")
                tut = postp.tile([P, NC5, IMG], BF16, tag="tut")
                tvt = postp.tile([P, NC5, IMG], BF16, tag="tvt")
                for k in range(NC5):
                    wk = O[k + 1] - O[k]
                    pa = ps2.tile([P, IMG], F32, tag="pa")
                    pb = ps2.tile([P, IMG], F32, tag="pb")
                    pu = ps2.tile([P, IMG], F32, tag="pu")
                    pv = ps2.tile([P, IMG], F32, tag="pv")
                    bmu, _ = band_ap(1, k)
                    bph, _ = band_ap(2, k)
                    bnh, _ = band_ap(3, k)
                    nc.tensor.matmul(pa[:wk, :], bmu, zt["zP"][:, k, :], start=True, stop=True)
                    nc.tensor.matmul(pb[:wk, :], bmu, zt["zM"][:, k, :], start=True, stop=True)
                    nc.tensor.matmul(pu[:wk, :], bph, zt["zP2"][:, k, :], start=True, stop=False)
                    nc.tensor.matmul(pu[:wk, :], bph, zt["zM2"][:, k, :], start=False, stop=True)
                    nc.tensor.matmul(pv[:wk, :], bph, zt["zP2"][:, k, :], start=True, stop=False)
                    nc.tensor.matmul(pv[:wk, :], bnh, zt["zM2"][:, k, :], start=False, stop=True)

                    # readout: x,y squares + tu,tv (+K12 bias) on ACT
                    nc.scalar.activation(xt[:wk, k, :], pa[:wk, :], AF.Square)
                    nc.scalar.activation(yt[:wk, k, :], pb[:wk, :], AF.Square)
                    nc.scalar.activation(tut[:wk, k, :], pu[:wk, :], AF.Identity, bias=k12c[:wk, :])
                    nc.scalar.activation(tvt[:wk, k, :], pv[:wk, :], AF.Identity, bias=k12c[:wk, :])

                # ---- packed pointwise ssim on [128, 2560] bf16 (garbage in
                # partition rows O[k+1]-O[k]..128 of each chunk never reaches
                # the reduction: the final STT only reads [:wk]).
                al = postp.tile([P, NC5, IMG], BF16, tag="al")
                be = postp.tile([P, NC5, IMG], BF16, tag="be")
                xf = xt[:].rearrange("p c w -> p (c w)")
                yf = yt[:].rearrange("p c w -> p (c w)")
                nc.vector.scalar_tensor_tensor(
                    al[:].rearrange("p c w -> p (c w)"), xf, C1, yf, OP.add, OP.subtract)
                nc.vector.scalar_tensor_tensor(
                    be[:].rearrange("p c w -> p (c w)"), xf, C1, yf, OP.add, OP.add)
                # gamma -> xt tile, delta -> yt tile (x,y dead now)
                ga = postp.tile([P, NC5, IMG], BF16, tag="xt")
                de = postp.tile([P, NC5, IMG], BF16, tag="yt")
                nc.vector.scalar_tensor_tensor(
                    ga[:].rearrange("p c w -> p (c w)"),
                    al[:].rearrange("p c w -> p (c w)"), -1.0,
                    tvt[:].rearrange("p c w -> p (c w)"), OP.mult, OP.add)
                nc.vector.scalar_tensor_tensor(
                    de[:].rearrange("p c w -> p (c w)"),
                    be[:].rearrange("p c w -> p (c w)"), -1.0,
                    tut[:].rearrange("p c w -> p (c w)"), OP.mult, OP.add)
                # num -> tvt tile; den f32 (reciprocal_approx needs fp32)
                nu = postp.tile([P, NC5, IMG], BF16, tag="tvt")
                dn = postp.tile([P, NC5, IMG], F32, tag="dnf")
                nc.vector.tensor_tensor(
                    nu[:].rearrange("p c w -> p (c w)"),
                    al[:].rearrange("p c w -> p (c w)"),
                    ga[:].rearrange("p c w -> p (c w)"), OP.mult)
                nc.vector.tensor_tensor(
                    dn[:].rearrange("p c w -> p (c w)"),
                    be[:].rearrange("p c w -> p (c w)"),
                    de[:].rearrange("p c w -> p (c w)"), OP.mult)
                rc = postp.tile([P, NC5, IMG], F32, tag="rcf")
                nc.vector.reciprocal_approx_fast(
                    rc[:].rearrange("p c w -> p (c w)"),
                    dn[:].rearrange("p c w -> p (c w)"))
                # fused ssim = num*rc with accumulation into partials
                jk = postp.tile([P, NC5, IMG], BF16, tag="be")
                for k in range(NC5):
                    wk = O[k + 1] - O[k]
                    nc.vector.scalar_tensor_tensor(
                        jk[:wk, k, :], nu[:wk, k, :], 1.0, rc[:wk, k, :],
                        OP.mult, OP.mult,
                        accum_out=partials[:wk, b * NC5 + k: b * NC5 + k + 1],
                    )

            final = accp.tile([P, 1], F32)
            nc.vector.tensor_reduce(final[:], partials[:], mybir.AxisListType.X, OP.add)
            nc.sync.dma_start(out_d[:], final[:])

    nc.compile()
    _CACHE["nc"] = nc
    return nc


def kernel(seg: np.ndarray, edge: np.ndarray) -> np.ndarray:
    nc = _build()
    seg = np.ascontiguousarray(seg, dtype=np.float32).reshape(N_CORES, PER_CORE, IMG, IMG)
    edge = np.ascontiguousarray(edge, dtype=np.float32).reshape(N_CORES, PER_CORE, IMG, IMG)
    in_maps = [{"seg": seg[c], "edge": edge[c]} for c in range(N_CORES)]
    res = run_bass_kernel_spmd(nc, in_maps, list(range(N_CORES)))
    total = 0.0
    for c in range(N_CORES):
        total += float(res.results[c]["out"].astype(np.float64).sum())
    mssim = total / (32.0 * IMG * IMG)
    return np.float32(1.0 - (1.0 + mssim) / 2.0)


# revision 9
# speedup vs baseline: 1.0973x; 1.0973x over previous
"""SSIM-based loss kernel for Trainium2 (8 NeuronCores, data-parallel over batch).

Computes: loss = 1 - (1 + mean(SSIM(sigmoid(seg), sigmoid(edge)))) / 2
for seg, edge of shape [32, 1, 512, 512] fp32, SSIM with a 7x7 gaussian
window (sigma=1.5), SAME zero-padding, C1=0.01^2, C2=0.03^2.

Sharding: batch dim across 8 cores (4 images each). Each core returns
per-partition partial sums of the ssim map; the host reduces and forms the
scalar loss.

v3 notes (HW-calibrated): DVE tensor_tensor bf16 SBUF runs 2x, STT runs 1x,
ACT is 1x with ~(fix+FD)/1.2GHz cost, PSUM reads are 1x. So the pointwise
chain is built from TT ops on bf16 with constants folded into ACT bias at
PSUM readout. Step-1/step-2 PSUM tiles are bank-paired so each readout
instruction covers two maps (FD=1024), halving fixed costs. The final
multiply+reduce is a fused STT with accum_out. den products run on the
otherwise-idle GPSIMD engine.

Math (per pixel, after 7x7 gaussian blur E[.]):
  pa = (mu1+mu2)/sqrt2, pb = (mu1-mu2)/sqrt2   [blur pipes of P=s+e, M=s-e]
  pu = E[s^2]+E[e^2]  (from (blur(P^2)+blur(M^2))/2)
  pv = 2 E[se]        (from (blur(P^2)-blur(M^2))/2)
  x = pa^2, y = pb^2;  w1 = x-y = 2 mu1 mu2;  w2 = x+y = mu1^2+mu2^2
  tv = pv + C2, tu = pu + C2
  gamma = tv - w1 (= 2 sigma12 + C2),  delta = tu - w2 (= sig1^2+sig2^2+C2)
  num = (w1+C1)*gamma,  den = (w2+C1)*delta,  ssim = num/den
"""

import numpy as np
import ml_dtypes

import concourse.bass as bass
import concourse.bacc as bacc
import concourse.tile as tile
import concourse.mybir as mybir
from concourse.bass_utils import run_bass_kernel_spmd

WS = 7
HW = WS // 2
SIGMA = 1.5
C1 = 0.01 ** 2
C2 = 0.03 ** 2

N_CORES = 8
IMG = 512
P = 128
PER_CORE = 4

# halo chunking: out regions [O[c], O[c+1]), input rows [R[c], R[c]+128)
O = [0, 122, 244, 366, 488, 512]
R = [0, 119, 241, 363, 384]
NC5 = 5

F32 = mybir.dt.float32
BF16 = mybir.dt.bfloat16
AF = mybir.ActivationFunctionType
OP = mybir.AluOpType
BF = ml_dtypes.bfloat16

GP_DEN = False  # GPSIMD float TT unsupported on HW (integer/power only)


def _gauss():
    x = np.arange(WS, dtype=np.float64)
    g = np.exp(-((x - HW) ** 2) / (2.0 * SIGMA ** 2))
    return g / g.sum()


def _band_tiles(scale):
    g = _gauss() * scale
    tiles = []
    for c in range(NC5):
        w = O[c + 1] - O[c]
        t = np.zeros((P, w), dtype=np.float64)
        for r in range(P):
            i = R[c] + r
            for j in range(w):
                d = (O[c] + j) - i
                if -HW <= d <= HW:
                    t[r, j] = g[d + HW]
        tiles.append(t.astype(np.float32))
    return tiles


_CACHE = {}


def _build():
    if "nc" in _CACHE:
        return _CACHE["nc"]

    nc = bacc.Bacc(None)

    seg_d = nc.dram_tensor("seg", [PER_CORE, IMG, IMG], F32, kind="ExternalInput")
    edge_d = nc.dram_tensor("edge", [PER_CORE, IMG, IMG], F32, kind="ExternalInput")
    out_d = nc.dram_tensor("out", [P, 1], F32, kind="ExternalOutput")

    # Band variants: 0: step1 (scale 1); 1: mu pipes (1/sqrt2); 2: +1/2; 3: -1/2
    variants = [1.0, 1.0 / np.sqrt(2.0), 0.5, -0.5]
    packed, offsets = [], []
    col = 0
    for v in variants:
        offs = []
        for t in _band_tiles(v):
            offs.append((col, t.shape[1]))
            packed.append(t)
            col += t.shape[1]
        offsets.append(offs)
    band_np = np.concatenate(packed, axis=1).astype(BF)  # [128, 2048] bf16
    band_d = nc.inline_tensor(band_np, name="band")

    # chunk pairs for FD=1024 ops: (0,1), (2,3), (4,)
    PAIRS = [(0, 2), (2, 2), (4, 1)]

    with tile.TileContext(nc) as tc:
        with (
            tc.tile_pool(name="const", bufs=1) as constp,
            tc.tile_pool(name="io", bufs=2) as iop,
            tc.tile_pool(name="sig", bufs=2) as sigp,
            tc.tile_pool(name="maps", bufs=2) as mapp,
            tc.tile_pool(name="zmaps", bufs=2) as zp,
            tc.tile_pool(name="ro", bufs=2) as rop,
            tc.tile_pool(name="chain", bufs=1) as chp,
            tc.tile_pool(name="acc", bufs=1) as accp,
            tc.tile_pool(name="psz", bufs=1, space="PSUM") as psz,
            tc.tile_pool(name="ps2", bufs=1, space="PSUM") as ps2,
        ):
            band = constp.tile([P, band_np.shape[1]], BF16)
            nc.sync.dma_start(band[:], band_d[:])

            def band_ap(v, c):
                c0, w = offsets[v][c]
                return band[:, c0:c0 + w], w

            partials = accp.tile([P, PER_CORE * 3], F32)
            nc.vector.memset(partials[:], 0.0)
            c2c = constp.tile([P, 1], F32)
            nc.vector.memset(c2c[:], C2)

            for b in range(PER_CORE):
                # DMA + sigmoid per chunk-pair (overlaps load with compute)
                st = sigp.tile([P, NC5, IMG], BF16, tag="st")
                et = sigp.tile([P, NC5, IMG], BF16, tag="et")
                for c0, w in PAIRS:
                    sgp_t = iop.tile([P, 2, IMG], F32, tag="sgp")
                    edp_t = iop.tile([P, 2, IMG], F32, tag="edp")
                    for i in range(w):
                        c = c0 + i
                        nc.sync.dma_start(sgp_t[:, i, :], seg_d[b, R[c]:R[c] + P, :])
                        nc.sync.dma_start(edp_t[:, i, :], edge_d[b, R[c]:R[c] + P, :])
                    nc.scalar.activation(st[:, c0:c0 + w, :], sgp_t[:, :w, :], AF.Sigmoid)
                    nc.scalar.activation(et[:, c0:c0 + w, :], edp_t[:, :w, :], AF.Sigmoid)

                sf = st[:].rearrange("p c w -> p (c w)")
                ef = et[:].rearrange("p c w -> p (c w)")
                Pt = mapp.tile([P, NC5, IMG], BF16, tag="P")
                Mt = mapp.tile([P, NC5, IMG], BF16, tag="M")
                nc.vector.tensor_tensor(Pt[:].rearrange("p c w -> p (c w)"), sf, ef, OP.add)
                nc.vector.tensor_tensor(Mt[:].rearrange("p c w -> p (c w)"), sf, ef, OP.subtract)
                P2t = mapp.tile([P, NC5, IMG], BF16, tag="P2")
                M2t = mapp.tile([P, NC5, IMG], BF16, tag="M2")
                pf = Pt[:].rearrange("p c w -> p (c w)")
                mf = Mt[:].rearrange("p c w -> p (c w)")
                nc.vector.tensor_tensor(P2t[:].rearrange("p c w -> p (c w)"), pf, pf, OP.mult)
                nc.vector.tensor_tensor(M2t[:].rearrange("p c w -> p (c w)"), mf, mf, OP.mult)

                # ---- blur step 1: z[w, j] (transposed, halo layout along w).
                # PSUM bank-paired: [zP|zM] copied by ACT, [zP2|zM2] by DVE.
                zPM = zp.tile([P, NC5, 2, IMG], BF16, tag="zPM")
                z22 = zp.tile([P, NC5, 2, IMG], BF16, tag="z22")
                for k in range(NC5):
                    pzPM = psz.tile([P, 2, IMG], F32, tag="pzPM")
                    pz22 = psz.tile([P, 2, IMG], F32, tag="pz22")
                    for half, src in ((0, Pt), (1, Mt)):
                        for c in range(NC5):
                            rhs, w = band_ap(0, c)
                            nc.tensor.matmul(
                                pzPM[:, half, O[c]:O[c + 1]],
                                src[:, c, R[k]:R[k] + P], rhs,
                                start=(c == 0), stop=(c == NC5 - 1))
                    for half, src in ((0, P2t), (1, M2t)):
                        for c in range(NC5):
                            rhs, w = band_ap(0, c)
                            nc.tensor.matmul(
                                pz22[:, half, O[c]:O[c + 1]],
                                src[:, c, R[k]:R[k] + P], rhs,
                                start=(c == 0), stop=(c == NC5 - 1))
                    nc.scalar.copy(zPM[:, k, :, :], pzPM[:])
                    nc.vector.tensor_copy(z22[:, k, :, :], pz22[:])

                # ---- blur step 2 (bank-paired [pa|pb], [pu|pv]) + readout
                xy = rop.tile([P, NC5, 2, IMG], BF16, tag="xy")
                tuv = rop.tile([P, NC5, 2, IMG], BF16, tag="tuv")
                for k in range(NC5):
                    wk = O[k + 1] - O[k]
                    pab = ps2.tile([P, 2, IMG], F32, tag="pab")
                    puv = ps2.tile([P, 2, IMG], F32, tag="puv")
                    bmu, _ = band_ap(1, k)
                    bph, _ = band_ap(2, k)
                    bnh, _ = band_ap(3, k)
                    nc.tensor.matmul(pab[:wk, 0, :], bmu, zPM[:, k, 0, :], start=True, stop=True)
                    nc.tensor.matmul(pab[:wk, 1, :], bmu, zPM[:, k, 1, :], start=True, stop=True)
                    nc.tensor.matmul(puv[:wk, 0, :], bph, z22[:, k, 0, :], start=True, stop=False)
                    nc.tensor.matmul(puv[:wk, 0, :], bph, z22[:, k, 1, :], start=False, stop=True)
                    nc.tensor.matmul(puv[:wk, 1, :], bph, z22[:, k, 0, :], start=True, stop=False)
                    nc.tensor.matmul(puv[:wk, 1, :], bnh, z22[:, k, 1, :], start=False, stop=True)
                    # x,y = pa^2,pb^2 ; tu,tv = pu,pv + C2  (FD=1024 each)
                    nc.scalar.activation(xy[:wk, k, :, :], pab[:wk, :, :], AF.Square)
                    nc.scalar.activation(tuv[:wk, k, :, :], puv[:wk, :, :], AF.Identity, bias=c2c[:wk, :])

                # ---- pointwise chain, TT-heavy (bf16 2x). Strided slices of
                # xy/tuv address the per-map planes. Garbage partition rows
                # (wk..128 of each chunk) never reach the reduction.
                xs = xy[:, :, 0, :]
                ys = xy[:, :, 1, :]
                tus = tuv[:, :, 0, :]
                tvs = tuv[:, :, 1, :]
                w1 = chp.tile([P, NC5, IMG], BF16, tag="w1")
                w2 = chp.tile([P, NC5, IMG], BF16, tag="w2")
                nc.vector.tensor_tensor(w1[:], xs, ys, OP.subtract)
                nc.vector.tensor_tensor(w2[:], xs, ys, OP.add)
                ga = chp.tile([P, NC5, IMG], BF16, tag="ga")
                de = chp.tile([P, NC5, IMG], BF16, tag="de")
                nc.vector.tensor_tensor(ga[:], tvs, w1[:], OP.subtract)
                nc.vector.tensor_tensor(de[:], tus, w2[:], OP.subtract)
                # num = (w1+C1)*gamma (STT 1x); den = (w2+C1)*delta
                nu = chp.tile([P, NC5, IMG], BF16, tag="nu")
                dn = chp.tile([P, NC5, IMG], F32, tag="dn")
                nc.vector.scalar_tensor_tensor(nu[:], w1[:], C1, ga[:], OP.add, OP.mult)
                if GP_DEN:
                    nc.gpsimd.scalar_tensor_tensor(dn[:], w2[:], C1, de[:], OP.add, OP.mult)
                else:
                    nc.vector.scalar_tensor_tensor(dn[:], w2[:], C1, de[:], OP.add, OP.mult)
                rc = chp.tile([P, NC5, IMG], F32, tag="rc")
                nc.vector.reciprocal_approx_fast(
                    rc[:].rearrange("p c w -> p (c w)"),
                    dn[:].rearrange("p c w -> p (c w)"))
                # fused ssim = num*rc with accumulation, per chunk-pair
                jk = chp.tile([P, NC5, IMG], BF16, tag="ga")
                for pi, (c0, w) in enumerate(PAIRS):
                    wk = O[c0 + 1] - O[c0]
                    nc.vector.scalar_tensor_tensor(
                        jk[:wk, c0:c0 + w, :], nu[:wk, c0:c0 + w, :], 1.0,
                        rc[:wk, c0:c0 + w, :], OP.mult, OP.mult,
                        accum_out=partials[:wk, b * 3 + pi: b * 3 + pi + 1],
                    )

            final = accp.tile([P, 1], F32)
            nc.vector.tensor_reduce(final[:], partials[:], mybir.AxisListType.X, OP.add)
            nc.sync.dma_start(out_d[:], final[:])

    nc.compile()
    _CACHE["nc"] = nc
    return nc


def kernel(seg: np.ndarray, edge: np.ndarray) -> np.ndarray:
    nc = _build()
    seg = np.ascontiguousarray(seg, dtype=np.float32).reshape(N_CORES, PER_CORE, IMG, IMG)
    edge = np.ascontiguousarray(edge, dtype=np.float32).reshape(N_CORES, PER_CORE, IMG, IMG)
    in_maps = [{"seg": seg[c], "edge": edge[c]} for c in range(N_CORES)]
    res = run_bass_kernel_spmd(nc, in_maps, list(range(N_CORES)))
    total = 0.0
    for c in range(N_CORES):
        total += float(res.results[c]["out"].astype(np.float64).sum())
    mssim = total / (32.0 * IMG * IMG)
    return np.float32(1.0 - (1.0 + mssim) / 2.0)
